# revision 1
# baseline (speedup 1.0000x reference)
"""GQA attention with LoRA-Q, tensor-parallel over 8 TRN2 cores.

Sharding (per core i of 8):
  - Q heads 4i..4i+3 (256 q-dims) and KV head i (GQA: repeat_interleave maps
    q heads [4i,4i+4) exactly onto kv head i).
  - Wq (with LoRA folded: Wq_eff = Wq + lora_B @ lora_A), Wk, Wv row-sharded;
    Wo column-sharded on its input (head) dim.
  - Attention outputs (transposed layout [hd, T]) are AllGathered, then each
    core computes a 256-column slice of the final output.

All matmuls in bf16 with fp32 PSUM accumulation; softmax without max
subtraction (scores are bounded: |S/8| <= ~7), denominator fused into the
PV matmul via an appended ones-column on V.
"""

import numpy as np
import ml_dtypes

import concourse.bass as bass
import concourse.mybir as mybir
import concourse.tile as tile
from concourse import bacc
from concourse.bass_utils import run_bass_kernel_spmd
from concourse.masks import make_identity

BF16 = mybir.dt.bfloat16
F32 = mybir.dt.float32

N_CORES = 8
T = 2048
D = 2048
HD = 64          # head dim
NH = 32          # total q heads
NKV = 8          # total kv heads
NH_LOC = NH // N_CORES       # 4 q heads per core
QW = NH_LOC * HD             # 256 q dims per core
P = 128
KT = D // P                  # 16 contraction tiles
CH = 512                     # T-chunk (psum free dim)
NCH = T // CH                # 4 chunks
NJ = T // P                  # 16 k-blocks
SCALE = 1.0 / 8.0            # 1/sqrt(64)


STOP_AFTER = None  # sim-bisect hook: "proj" | "rope" | "attn" | "norm"


def build_bass(st_group: int = 2):
    nc = bacc.Bacc(None, num_devices=N_CORES)

    # I/O
    xT_d = nc.dram_tensor("xT", [D, T], BF16, kind="ExternalInput")
    w_d = nc.dram_tensor("w_all", [D, QW + 2 * HD], BF16, kind="ExternalInput")
    woT_d = nc.dram_tensor("woT", [D, QW], BF16, kind="ExternalInput")
    cos2_d = nc.dram_tensor("cos2", [P, T], BF16, kind="ExternalInput")
    sin2_d = nc.dram_tensor("sin2", [P, T], BF16, kind="ExternalInput")
    mask_d = nc.dram_tensor("dmask", [P, 4, CH], BF16, kind="ExternalInput")
    y_d = nc.dram_tensor("y", [T, QW], F32, kind="ExternalOutput")

    with tile.TileContext(nc, num_cores=N_CORES) as tc:
        _body(nc, tc, xT_d, w_d, woT_d, cos2_d, sin2_d, mask_d, y_d, st_group)
    nc.compile()
    return nc


def _body(nc, tc, xT_d, w_d, woT_d, cos2_d, sin2_d, mask_d, y_d, st_group):
    import contextlib

    ctx = contextlib.ExitStack()
    with ctx:
        consts = ctx.enter_context(tc.tile_pool(name="consts", bufs=1))
        big = ctx.enter_context(tc.tile_pool(name="big", bufs=1))
        work = ctx.enter_context(tc.tile_pool(name="work", bufs=1))
        rope_p = ctx.enter_context(tc.tile_pool(name="rope_p", bufs=1))
        pt_p = ctx.enter_context(tc.tile_pool(name="pt_p", bufs=3))
        rcp_p = ctx.enter_context(tc.tile_pool(name="rcp_p", bufs=2))
        psum_st = ctx.enter_context(tc.tile_pool(name="psum_st", bufs=2, space="PSUM"))
        psum_o = ctx.enter_context(tc.tile_pool(name="psum_o", bufs=2, space="PSUM"))
        dram = ctx.enter_context(tc.tile_pool(name="dram", bufs=1, space="DRAM"))

        # ---- constants (large loads split per-kt: one dma_start rides a
        # single DMA engine at ~22 GB/s, so chunking is what buys bandwidth)
        w_sb = consts.tile([P, KT, QW + 2 * HD], BF16)
        w_r = w_d.rearrange("(kt p) m -> p kt m", p=P)
        for kt in range(KT):
            nc.sync.dma_start(w_sb[:, kt, :], w_r[:, kt, :])
        woT_sb = consts.tile([P, KT, QW], BF16)
        woT_r = woT_d.rearrange("(kt p) m -> p kt m", p=P)
        for kt in range(KT):
            nc.sync.dma_start(woT_sb[:, kt, :], woT_r[:, kt, :])
        cos2_sb = consts.tile([P, T], BF16)
        nc.sync.dma_start(cos2_sb, cos2_d[:])
        sin2_sb = consts.tile([P, T], BF16)
        nc.sync.dma_start(sin2_sb, sin2_d[:])
        mask_sb = consts.tile([P, 4, CH], BF16)
        nc.sync.dma_start(mask_sb, mask_d[:])
        ident64 = consts.tile([HD, HD], BF16)
        make_identity(nc, ident64)
        ones64 = consts.tile([1, HD], BF16)
        nc.vector.memset(ones64, 1.0)

        # v with ones column appended: [tk(P), j, HD+1]
        v_aug = work.tile([P, NJ, HD + 1], BF16)
        nc.vector.memset(v_aug[:, :, HD : HD + 1], 1.0)

        # ---- load xT resident (32 chunked DMAs across queues)
        xT_sb = big.tile([P, KT, T], BF16, tag="big", name="xT_sb")
        xT_r = xT_d.rearrange("(kt p) t -> p kt t", p=P)
        for kt in range(KT):
            for half in range(2):
                sl = slice(half * (T // 2), (half + 1) * (T // 2))
                nc.sync.dma_start(xT_sb[:, kt, sl], xT_r[:, kt, sl])

        # ---- fused QKV projection (transposed layout): projT[m] rows =
        # [q heads 2m, 2m+1] for m in {0,1}; m=2 rows 0:64 = kT, 64:128 = vT
        projT = work.tile([P, 3, T], BF16)
        for m in range(3):
            for c in range(NCH):
                ps = psum_o.tile([P, CH], F32, tag="mm")
                for kt in range(KT):
                    nc.tensor.matmul(
                        ps,
                        lhsT=w_sb[:, kt, m * P : (m + 1) * P],
                        rhs=xT_sb[:, kt, c * CH : (c + 1) * CH],
                        start=(kt == 0),
                        stop=(kt == KT - 1),
                    )
                nc.vector.tensor_copy(projT[:, m, c * CH : (c + 1) * CH], ps)

        if STOP_AFTER == "proj":
            nc.gpsimd.dma_start(y_d[0:P, :], projT[:, 0, 0:QW])
            return
        # ---- RoPE on q head-pairs -> qT_sb [64, 4, T] (head-major, base 0)
        qT_sb = work.tile([HD, NH_LOC, T], BF16)
        for s in range(2):
            src = projT[:, s, :]
            shuf = rope_p.tile([P, T], BF16, tag="shuf")
            for (a, b) in ((0, 32), (32, 0), (64, 96), (96, 64)):
                nc.sync.dma_start(shuf[a : a + 32, :], src[b : b + 32, :])
            t1 = rope_p.tile([P, T], BF16, tag="t1")
            nc.vector.tensor_mul(t1, src, cos2_sb)
            t2 = rope_p.tile([P, T], BF16, tag="t2")
            nc.vector.tensor_mul(t2, shuf, sin2_sb)
            nc.vector.tensor_add(t1, t1, t2)
            nc.sync.dma_start(qT_sb[:, 2 * s, :], t1[0:HD, :])
            nc.sync.dma_start(qT_sb[:, 2 * s + 1, :], t1[HD:P, :])

        # ---- RoPE on k (rows 0:64 of projT[:,2]) -> kT_sb [64, T]
        kT_sb = work.tile([HD, T], BF16)
        ksrc = projT[0:HD, 2, :]
        kshuf = rope_p.tile([P, T], BF16, tag="shuf", name="kshuf")
        nc.sync.dma_start(kshuf[0:32, :], ksrc[32:HD, :])
        nc.sync.dma_start(kshuf[32:HD, :], ksrc[0:32, :])
        kt1 = rope_p.tile([P, T], BF16, tag="t1", name="kt1")
        nc.vector.tensor_mul(kt1[0:HD, :], ksrc, cos2_sb[0:HD, :])
        kt2 = rope_p.tile([P, T], BF16, tag="t2", name="kt2")
        nc.vector.tensor_mul(kt2[0:HD, :], kshuf[0:HD, :], sin2_sb[0:HD, :])
        nc.vector.tensor_add(kT_sb, kt1[0:HD, :], kt2[0:HD, :])

        # ---- transpose v: vT (projT[64:128, 2]) -> v_aug[:, j, 0:64]
        vT0 = work.tile([HD, T], BF16)
        nc.sync.dma_start(vT0, projT[HD:P, 2, :])
        for j in range(NJ):
            tp = psum_o.tile([P, CH], BF16, tag="mm")
            nc.tensor.transpose(tp[:, 0:HD], vT0[:, j * P : (j + 1) * P], ident64)
            nc.vector.tensor_copy(v_aug[:, j, 0:HD], tp[:, 0:HD])

        if STOP_AFTER == "rope":
            nc.gpsimd.dma_start(y_d[0:HD, :], qT_sb[:, 0, 0:QW])
            return
        # ---- attention per local head, transposed-scores flash style
        # OT_stage rows 0:64 = unnormalized O^T (bf16), row 64 = denominator
        OT_stage = work.tile([HD + 1, NH_LOC, T], BF16)
        OT_sb = work.tile([HD, NH_LOC, T], BF16)
        ot_dram = dram.tile([QW, T], BF16)
        ot_r = ot_dram.rearrange("(h d) t -> d h t", h=NH_LOC)
        G = st_group
        for h in range(NH_LOC):
            for c in range(NCH):
                nj = 4 * c + 4          # causal: k-blocks 0..4c+3
                groups = [
                    list(range(g, min(g + G, nj))) for g in range(0, nj, G)
                ]
                ot = psum_o.tile([P, CH], F32, tag="ot")

                def do_st(js):
                    st = psum_st.tile([P, G, CH], F32, tag="st")
                    for idx, j in enumerate(js):
                        nc.tensor.matmul(
                            st[:, idx, :],
                            lhsT=kT_sb[:, j * P : (j + 1) * P],
                            rhs=qT_sb[:, h, c * CH : (c + 1) * CH],
                            start=True,
                            stop=True,
                        )
                    return st

                def do_rest(st, js):
                    n = len(js)
                    pt = pt_p.tile([P, G, CH], BF16, tag="pt")
                    nc.scalar.activation(
                        pt[:, 0:n, :], st[:, 0:n, :],
                        mybir.ActivationFunctionType.Exp, scale=SCALE,
                    )
                    for idx, j in enumerate(js):
                        if j >= 4 * c:  # diagonal block: zero masked region
                            nc.vector.tensor_mul(
                                pt[:, idx, :], pt[:, idx, :],
                                mask_sb[:, j - 4 * c, :],
                            )
                    for idx, j in enumerate(js):
                        nc.tensor.matmul(
                            ot[0 : HD + 1, :],
                            lhsT=v_aug[:, j, :],
                            rhs=pt[:, idx, :],
                            start=(j == 0),
                            stop=(j == nj - 1),
                            skip_group_check=True,
                        )

                # software-pipeline: issue ST of group g+1 before PV of g
                st_cur = do_st(groups[0])
                for g in range(len(groups)):
                    st_next = do_st(groups[g + 1]) if g + 1 < len(groups) else None
                    do_rest(st_cur, groups[g])
                    st_cur = st_next

                nc.vector.tensor_copy(
                    OT_stage[:, h, c * CH : (c + 1) * CH], ot[0 : HD + 1, :]
                )

            # per-head softmax normalization (overlaps next head's attention)
            den_h = rcp_p.tile([NCH, CH], BF16, tag="den")
            recip_h = rcp_p.tile([NCH, CH], BF16, tag="recip")
            for c in range(NCH):
                nc.sync.dma_start(
                    den_h[c : c + 1, :],
                    OT_stage[HD : HD + 1, h, c * CH : (c + 1) * CH],
                )
            with nc.allow_low_precision("softmax denom in bf16 is fine"):
                nc.vector.reciprocal(recip_h, den_h)
            for c in range(NCH):
                rrow = rcp_p.tile([1, CH], BF16, tag="rrow")
                nc.sync.dma_start(rrow, recip_h[c : c + 1, :])
                bc = psum_o.tile([P, CH], F32, tag="mm")
                nc.tensor.matmul(
                    bc[0:HD, :], lhsT=ones64, rhs=rrow, start=True, stop=True
                )
                nc.vector.tensor_mul(
                    OT_sb[:, h, c * CH : (c + 1) * CH],
                    OT_stage[0:HD, h, c * CH : (c + 1) * CH],
                    bc[0:HD, :],
                )
            nc.sync.dma_start(ot_r[:, h, :], OT_sb[:, h, :])

        if STOP_AFTER == "attn":
            nc.gpsimd.dma_start(y_d[0 : HD + 1, :], OT_stage[:, 0, 0:QW])
            return
        if STOP_AFTER == "norm":
            nc.gpsimd.dma_start(y_d[0:HD, :], OT_sb[:, 0, 0:QW])
            return
        # ---- AllGather of O^T across cores -> [D(=NH*HD), T]
        ofull_dram = dram.tile([D, T], BF16, addr_space="Shared")
        nc.gpsimd.collective_compute(
            "AllGather",
            mybir.AluOpType.bypass,
            replica_groups=[list(range(N_CORES))],
            ins=[ot_dram.opt()],
            outs=[ofull_dram.opt()],
        )

        # ---- final projection: y[:, slice] = O_full @ Wo_slice^T
        ofull_sb = big.tile([P, KT, T], BF16, tag="big", name="ofull_sb")
        of_r = ofull_dram.rearrange("(kt p) t -> p kt t", p=P)
        for kt in range(KT):
            for half in range(2):
                sl = slice(half * (T // 2), (half + 1) * (T // 2))
                nc.sync.dma_start(ofull_sb[:, kt, sl], of_r[:, kt, sl])
        for mt in range(T // P):
            ps = psum_o.tile([P, CH], F32, tag="mm")
            for kt in range(KT):
                nc.tensor.matmul(
                    ps[:, 0:QW],
                    lhsT=ofull_sb[:, kt, mt * P : (mt + 1) * P],
                    rhs=woT_sb[:, kt, :],
                    start=(kt == 0),
                    stop=(kt == KT - 1),
                )
            y_sb = rcp_p.tile([P, QW], F32, tag="y_sb")
            nc.vector.tensor_copy(y_sb, ps[:, 0:QW])
            nc.sync.dma_start(y_d[mt * P : (mt + 1) * P, :], y_sb)


def _prep_shards(x, Wq, lora_A, lora_B, Wk, Wv, Wo):
    bf16 = ml_dtypes.bfloat16
    xT = np.ascontiguousarray(x[0].T).astype(bf16)

    theta = 1.0 / (10000.0 ** (np.arange(0, HD, 2, dtype=np.float32) / HD))
    pos = np.arange(T, dtype=np.float32)
    ang = pos[:, None] * theta[None, :]
    ang = np.concatenate([ang, ang], axis=-1)          # [T, HD]
    cosT = np.cos(ang).T                               # [HD, T]
    sinT = np.sin(ang).T
    sign = np.where(np.arange(HD) < HD // 2, -1.0, 1.0).astype(np.float32)
    sinTs = sinT * sign[:, None]
    cos2 = np.ascontiguousarray(np.concatenate([cosT, cosT], 0)).astype(bf16)
    sin2 = np.ascontiguousarray(np.concatenate([sinTs, sinTs], 0)).astype(bf16)

    p_idx = np.arange(P)[:, None, None]
    m_idx = np.arange(4)[None, :, None]
    f_idx = np.arange(CH)[None, None, :]
    dmask = (p_idx + P * m_idx <= f_idx).astype(bf16)  # [128, 4, 512]

    Wq_eff = Wq + lora_B.astype(np.float64) @ lora_A.astype(np.float64)
    Wq_eff = Wq_eff.astype(np.float32)

    in_maps = []
    for i in range(N_CORES):
        wq_i = Wq_eff[QW * i : QW * (i + 1), :]        # [256, D]
        wk_i = Wk[HD * i : HD * (i + 1), :]            # [64, D]
        wv_i = Wv[HD * i : HD * (i + 1), :]
        w_all = np.ascontiguousarray(
            np.concatenate([wq_i, wk_i, wv_i], 0).T
        ).astype(bf16)                                 # [D, 384]
        woT = np.ascontiguousarray(Wo[QW * i : QW * (i + 1), :].T).astype(bf16)
        in_maps.append({
            "xT": xT,
            "w_all": w_all,
            "woT": woT,
            "cos2": cos2,
            "sin2": sin2,
            "dmask": dmask,
        })
    return in_maps


def run(inputs, trace=False, **kw):
    nc = build_bass()
    in_maps = _prep_shards(**inputs)
    res = run_bass_kernel_spmd(
        nc, in_maps, core_ids=list(range(N_CORES)), trace=trace, **kw
    )
    y = np.concatenate([res.results[i]["y"] for i in range(N_CORES)], axis=1)
    return y[None].astype(np.float32), res


def kernel(**inputs):
    y, _ = run(inputs)
    return y



# revision 3
# speedup vs baseline: 1.7413x; 1.7413x over previous
"""GQA attention with LoRA-Q, tensor-parallel over 8 TRN2 cores.

Sharding (per core i of 8):
  - Q heads 4i..4i+3 (256 q-dims) and KV head i (GQA: repeat_interleave maps
    q heads [4i,4i+4) exactly onto kv head i).
  - Wq (with LoRA folded: Wq_eff = Wq + lora_B @ lora_A), Wk, Wv row-sharded;
    Wo column-sharded on its input (head) dim.
  - Each core computes a full-width PARTIAL output y_part = O_loc @ Wo_loc^T
    [T, D]; a ReduceScatter (add) over rows gives core i the final rows
    256i..256(i+1) — 8x less collective traffic than AllGathering O.

All matmuls in bf16 with fp32 PSUM accumulation; softmax without max
subtraction (scores are bounded: |S/8| <= ~7), denominator fused into the
PV matmul via an appended ones-column on V.
"""

import numpy as np
import ml_dtypes

import concourse.bass as bass
import concourse.mybir as mybir
import concourse.tile as tile
from concourse import bacc
from concourse.bass_utils import run_bass_kernel_spmd
from concourse.masks import make_identity

BF16 = mybir.dt.bfloat16
F32 = mybir.dt.float32

N_CORES = 8
T = 2048
D = 2048
HD = 64          # head dim
NH = 32          # total q heads
NKV = 8          # total kv heads
NH_LOC = NH // N_CORES       # 4 q heads per core
QW = NH_LOC * HD             # 256 q dims per core
P = 128
KT = D // P                  # 16 contraction tiles
CH = 512         # T-chunk (psum free dim)
NCH = T // CH                # 4 chunks
NJ = T // P                  # 16 k-blocks
SCALE = 1.0 / 8.0            # 1/sqrt(64)
TR = T // N_CORES            # 256 output rows per core after ReduceScatter


STOP_AFTER = None  # sim-bisect hook: "proj" | "rope" | "attn" | "norm"


def build_bass(st_group: int = 2):
    nc = bacc.Bacc(None, num_devices=N_CORES)

    # I/O
    xT_d = nc.dram_tensor("xT", [D, T], BF16, kind="ExternalInput")
    w_d = nc.dram_tensor("w_all", [D, QW + 2 * HD], BF16, kind="ExternalInput")
    woT_d = nc.dram_tensor("woT", [QW, D], BF16, kind="ExternalInput")
    cos2_d = nc.dram_tensor("cos2", [P, T], BF16, kind="ExternalInput")
    sin2_d = nc.dram_tensor("sin2", [P, T], BF16, kind="ExternalInput")
    mask_d = nc.dram_tensor("dmask", [P, 4, CH], BF16, kind="ExternalInput")
    y_d = nc.dram_tensor("y", [TR, D], BF16, kind="ExternalOutput")

    with tile.TileContext(nc, num_cores=N_CORES) as tc:
        _body(nc, tc, xT_d, w_d, woT_d, cos2_d, sin2_d, mask_d, y_d, st_group)
    nc.compile()
    return nc


def _body(nc, tc, xT_d, w_d, woT_d, cos2_d, sin2_d, mask_d, y_d, st_group):
    import contextlib

    ctx = contextlib.ExitStack()
    with ctx:
        consts = ctx.enter_context(tc.tile_pool(name="consts", bufs=1))
        big = ctx.enter_context(tc.tile_pool(name="big", bufs=1))
        work = ctx.enter_context(tc.tile_pool(name="work", bufs=1))
        rope_p = ctx.enter_context(tc.tile_pool(name="rope_p", bufs=1))
        pt_p = ctx.enter_context(tc.tile_pool(name="pt_p", bufs=3))
        rcp_p = ctx.enter_context(tc.tile_pool(name="rcp_p", bufs=2))
        psum_st = ctx.enter_context(tc.tile_pool(name="psum_st", bufs=2, space="PSUM"))
        psum_o = ctx.enter_context(tc.tile_pool(name="psum_o", bufs=2, space="PSUM"))
        dram = ctx.enter_context(tc.tile_pool(name="dram", bufs=1, space="DRAM"))

        # ---- constants: one dma_start each (HWDGE setup ~625ns is the per-DMA
        # cost that dominates; transfers themselves run at full bus speed)
        w_sb = consts.tile([P, KT, QW + 2 * HD], BF16)
        w_r = w_d.rearrange("(kt p) m -> p kt m", p=P)
        nc.sync.dma_start(w_sb, w_r)

        # ---- load xT resident; chunked (ktg x t-half) so proj can start early
        xT_sb = big.tile([P, KT, T], BF16, tag="big", name="xT_sb")
        xT_r = xT_d.rearrange("(kt p) t -> p kt t", p=P)
        for ktg in range(4):
            for half in range(2):
                sl = slice(half * (T // 2), (half + 1) * (T // 2))
                ks = slice(4 * ktg, 4 * ktg + 4)
                nc.sync.dma_start(xT_sb[:, ks, sl], xT_r[:, ks, sl])

        cos2_sb = consts.tile([P, T], BF16)
        nc.sync.dma_start(cos2_sb, cos2_d[:])
        sin2_sb = consts.tile([P, T], BF16)
        nc.sync.dma_start(sin2_sb, sin2_d[:])
        mask_sb = consts.tile([P, 4, CH], BF16)
        nc.sync.dma_start(mask_sb, mask_d[:])
        # Wo slice (transposed): [hd_loc=256, D] -> [p, kh, D]
        woT_sb = consts.tile([P, 2, D], BF16)
        woT_r = woT_d.rearrange("(kh p) d -> p kh d", p=P)
        nc.sync.dma_start(woT_sb, woT_r)

        ident64 = consts.tile([HD, HD], BF16)
        make_identity(nc, ident64)
        ones64 = consts.tile([1, HD], BF16)
        nc.vector.memset(ones64, 1.0)

        # v with ones column appended: [tk(P), j, HD+1]
        v_aug = work.tile([P, NJ, HD + 1], BF16)
        nc.vector.memset(v_aug[:, :, HD : HD + 1], 1.0)

        # ---- fused QKV projection (transposed layout): projT[m] rows =
        # [q heads 2m, 2m+1] for m in {0,1}; m=2 rows 0:64 = kT, 64:128 = vT
        projT = work.tile([P, 3, T], BF16)
        for m in range(3):
            for c in range(NCH):
                ps = psum_o.tile([P, CH], F32, tag="mm")
                for kt in range(KT):
                    nc.tensor.matmul(
                        ps,
                        lhsT=w_sb[:, kt, m * P : (m + 1) * P],
                        rhs=xT_sb[:, kt, c * CH : (c + 1) * CH],
                        start=(kt == 0),
                        stop=(kt == KT - 1),
                    )
                nc.vector.tensor_copy(projT[:, m, c * CH : (c + 1) * CH], ps)

        if STOP_AFTER == "proj":
            nc.gpsimd.dma_start(y_d[0:P, :], projT[:, 0, 0:D])
            return
        # ---- RoPE on q head-pairs -> qT_sb [64, 4, T] (head-major, base 0)
        qT_sb = work.tile([HD, NH_LOC, T], BF16)
        for s in range(2):
            src = projT[:, s, :]
            shuf = rope_p.tile([P, T], BF16, tag="shuf")
            for (a, b) in ((0, 32), (32, 0), (64, 96), (96, 64)):
                nc.sync.dma_start(shuf[a : a + 32, :], src[b : b + 32, :])
            t1 = rope_p.tile([P, T], BF16, tag="t1")
            nc.vector.tensor_mul(t1, src, cos2_sb)
            t2 = rope_p.tile([P, T], BF16, tag="t2")
            nc.vector.tensor_mul(t2, shuf, sin2_sb)
            nc.vector.tensor_add(t1, t1, t2)
            nc.sync.dma_start(qT_sb[:, 2 * s, :], t1[0:HD, :])
            nc.sync.dma_start(qT_sb[:, 2 * s + 1, :], t1[HD:P, :])

        # ---- RoPE on k (rows 0:64 of projT[:,2]) -> kT_sb [64, T]
        kT_sb = work.tile([HD, T], BF16)
        ksrc = projT[0:HD, 2, :]
        kshuf = rope_p.tile([P, T], BF16, tag="shuf", name="kshuf")
        nc.sync.dma_start(kshuf[0:32, :], ksrc[32:HD, :])
        nc.sync.dma_start(kshuf[32:HD, :], ksrc[0:32, :])
        kt1 = rope_p.tile([P, T], BF16, tag="t1", name="kt1")
        nc.vector.tensor_mul(kt1[0:HD, :], ksrc, cos2_sb[0:HD, :])
        kt2 = rope_p.tile([P, T], BF16, tag="t2", name="kt2")
        nc.vector.tensor_mul(kt2[0:HD, :], kshuf[0:HD, :], sin2_sb[0:HD, :])
        nc.vector.tensor_add(kT_sb, kt1[0:HD, :], kt2[0:HD, :])

        # ---- transpose v: vT (projT[64:128, 2]) -> v_aug[:, j, 0:64]
        vT0 = work.tile([HD, T], BF16)
        nc.sync.dma_start(vT0, projT[HD:P, 2, :])
        for j in range(NJ):
            tp = psum_o.tile([P, CH], BF16, tag="mm")
            nc.tensor.transpose(tp[:, 0:HD], vT0[:, j * P : (j + 1) * P], ident64)
            nc.vector.tensor_copy(v_aug[:, j, 0:HD], tp[:, 0:HD])

        if STOP_AFTER == "rope":
            nc.gpsimd.dma_start(y_d[0:HD, :], qT_sb[:, 0, 0:D])
            return
        # ---- attention per local head, transposed-scores flash style
        # OT_stage rows 0:64 = unnormalized O^T (bf16), row 64 = denominator
        OT_stage = work.tile([HD + 1, NH_LOC, T], BF16)
        # normalized O^T packed for the Wo matmul: row kh*128+p = local hd dim
        OT128 = work.tile([P, 2, T], BF16)
        G = st_group
        for h in range(NH_LOC):
            for c in range(NCH):
                nj = 4 * c + 4          # causal: k-blocks 0..4c+3
                groups = [
                    list(range(g, min(g + G, nj))) for g in range(0, nj, G)
                ]
                ot = psum_o.tile([P, CH], F32, tag="ot")

                def do_st(js):
                    st = psum_st.tile([P, G, CH], F32, tag="st")
                    for idx, j in enumerate(js):
                        nc.tensor.matmul(
                            st[:, idx, :],
                            lhsT=kT_sb[:, j * P : (j + 1) * P],
                            rhs=qT_sb[:, h, c * CH : (c + 1) * CH],
                            start=True,
                            stop=True,
                        )
                    return st

                def do_rest(st, js):
                    n = len(js)
                    pt = pt_p.tile([P, G, CH], BF16, tag="pt")
                    nc.scalar.activation(
                        pt[:, 0:n, :], st[:, 0:n, :],
                        mybir.ActivationFunctionType.Exp, scale=SCALE,
                    )
                    for idx, j in enumerate(js):
                        if j >= 4 * c:  # diagonal block: zero masked region
                            nc.vector.tensor_mul(
                                pt[:, idx, :], pt[:, idx, :],
                                mask_sb[:, j - 4 * c, :],
                            )
                    for idx, j in enumerate(js):
                        nc.tensor.matmul(
                            ot[0 : HD + 1, :],
                            lhsT=v_aug[:, j, :],
                            rhs=pt[:, idx, :],
                            start=(j == 0),
                            stop=(j == nj - 1),
                            skip_group_check=True,
                        )

                # software-pipeline: issue ST of group g+1 before PV of g
                st_cur = do_st(groups[0])
                for g in range(len(groups)):
                    st_next = do_st(groups[g + 1]) if g + 1 < len(groups) else None
                    do_rest(st_cur, groups[g])
                    st_cur = st_next

                nc.vector.tensor_copy(
                    OT_stage[:, h, c * CH : (c + 1) * CH], ot[0 : HD + 1, :]
                )

            # per-head softmax normalization (overlaps next head's attention)
            den_h = rcp_p.tile([NCH, CH], BF16, tag="den")
            recip_h = rcp_p.tile([NCH, CH], BF16, tag="recip")
            for c in range(NCH):
                nc.sync.dma_start(
                    den_h[c : c + 1, :],
                    OT_stage[HD : HD + 1, h, c * CH : (c + 1) * CH],
                )
            with nc.allow_low_precision("softmax denom in bf16 is fine"):
                nc.vector.reciprocal(recip_h, den_h)
            hp = (h % 2) * HD       # partition base within OT128
            kh = h // 2
            for c in range(NCH):
                rrow = rcp_p.tile([1, CH], BF16, tag="rrow")
                nc.sync.dma_start(rrow, recip_h[c : c + 1, :])
                bc = psum_o.tile([P, CH], F32, tag="mm")
                nc.tensor.matmul(
                    bc[0:HD, :], lhsT=ones64, rhs=rrow, start=True, stop=True
                )
                nc.vector.tensor_mul(
                    OT128[hp : hp + HD, kh, c * CH : (c + 1) * CH],
                    OT_stage[0:HD, h, c * CH : (c + 1) * CH],
                    bc[0:HD, :],
                )

        if STOP_AFTER == "attn":
            nc.gpsimd.dma_start(y_d[0 : HD + 1, :], OT_stage[:, 0, 0:D])
            return
        if STOP_AFTER == "norm":
            nc.gpsimd.dma_start(y_d[0:HD, :], OT128[:, 0, 0:D])
            return
        # ---- partial output projection: y_part[t, d] = O_loc @ Wo_loc^T,
        # then one ReduceScatter(add) over rows -> y rows 256i..256(i+1).
        ypart_sb = big.tile([P, KT, D], BF16, tag="big", name="ypart_sb")
        ypart_dram = dram.tile([T, D], BF16)
        ypart_r = ypart_dram.rearrange("(mt p) d -> p mt d", p=P)
        for mtg in range(4):
            for mt in range(4 * mtg, 4 * mtg + 4):
                for dc in range(NCH):
                    ps = psum_o.tile([P, CH], F32, tag="mm")
                    for kh in range(2):
                        nc.tensor.matmul(
                            ps,
                            lhsT=OT128[:, kh, mt * P : (mt + 1) * P],
                            rhs=woT_sb[:, kh, dc * CH : (dc + 1) * CH],
                            start=(kh == 0),
                            stop=(kh == 1),
                        )
                    if dc % 2 == 0:
                        nc.vector.tensor_copy(
                            ypart_sb[:, mt, dc * CH : (dc + 1) * CH], ps
                        )
                    else:
                        nc.scalar.copy(
                            ypart_sb[:, mt, dc * CH : (dc + 1) * CH], ps
                        )
            ms = slice(4 * mtg, 4 * mtg + 4)
            nc.sync.dma_start(ypart_r[:, ms, :], ypart_sb[:, ms, :])

        y_rs = dram.tile([TR, D], BF16)
        nc.gpsimd.collective_compute(
            "ReduceScatter",
            mybir.AluOpType.add,
            replica_groups=[list(range(N_CORES))],
            ins=[ypart_dram[:]],
            outs=[y_rs[:]],
        )
        nc.sync.dma_start(y_d[:], y_rs[:])


def _prep_shards(x, Wq, lora_A, lora_B, Wk, Wv, Wo):
    bf16 = ml_dtypes.bfloat16
    xT = np.ascontiguousarray(x[0].T).astype(bf16)

    theta = 1.0 / (10000.0 ** (np.arange(0, HD, 2, dtype=np.float32) / HD))
    pos = np.arange(T, dtype=np.float32)
    ang = pos[:, None] * theta[None, :]
    ang = np.concatenate([ang, ang], axis=-1)          # [T, HD]
    cosT = np.cos(ang).T                               # [HD, T]
    sinT = np.sin(ang).T
    sign = np.where(np.arange(HD) < HD // 2, -1.0, 1.0).astype(np.float32)
    sinTs = sinT * sign[:, None]
    cos2 = np.ascontiguousarray(np.concatenate([cosT, cosT], 0)).astype(bf16)
    sin2 = np.ascontiguousarray(np.concatenate([sinTs, sinTs], 0)).astype(bf16)

    p_idx = np.arange(P)[:, None, None]
    m_idx = np.arange(4)[None, :, None]
    f_idx = np.arange(CH)[None, None, :]
    dmask = (p_idx + P * m_idx <= f_idx).astype(bf16)  # [128, 4, 512]

    Wq_eff = Wq + lora_B.astype(np.float64) @ lora_A.astype(np.float64)
    Wq_eff = Wq_eff.astype(np.float32)

    in_maps = []
    for i in range(N_CORES):
        wq_i = Wq_eff[QW * i : QW * (i + 1), :]        # [256, D]
        wk_i = Wk[HD * i : HD * (i + 1), :]            # [64, D]
        wv_i = Wv[HD * i : HD * (i + 1), :]
        w_all = np.ascontiguousarray(
            np.concatenate([wq_i, wk_i, wv_i], 0).T
        ).astype(bf16)                                 # [D, 384]
        # Wo columns for this core's heads, transposed: [256, D]
        woT = np.ascontiguousarray(Wo[:, QW * i : QW * (i + 1)].T).astype(bf16)
        in_maps.append({
            "xT": xT,
            "w_all": w_all,
            "woT": woT,
            "cos2": cos2,
            "sin2": sin2,
            "dmask": dmask,
        })
    return in_maps


def run(inputs, trace=False, **kw):
    nc = build_bass()
    in_maps = _prep_shards(**inputs)
    res = run_bass_kernel_spmd(
        nc, in_maps, core_ids=list(range(N_CORES)), trace=trace, **kw
    )
    y = np.concatenate(
        [np.asarray(res.results[i]["y"]).astype(np.float32) for i in range(N_CORES)],
        axis=0,
    )
    return y[None], res


def kernel(**inputs):
    y, _ = run(inputs)
    return y


# revision 18
# speedup vs baseline: 2.0357x; 1.1691x over previous
"""GQA attention with LoRA-Q, tensor-parallel over 8 TRN2 cores.

Sharding (per core i of 8):
  - Q heads 4i..4i+3 (256 q-dims) and KV head i (GQA: repeat_interleave maps
    q heads [4i,4i+4) exactly onto kv head i).
  - Wq (with LoRA folded: Wq_eff = Wq + lora_B @ lora_A), Wk, Wv row-sharded;
    Wo column-sharded on its input (head) dim.
  - Each core computes a full-width PARTIAL output y_part = O_loc @ Wo_loc^T
    [T, D]; per-T-chunk ReduceScatter(add) over rows gives core i rows
    512c+64i..+64 — 8x less collective traffic than AllGathering O, and the
    first three collectives overlap attention compute.

Single fused pipeline over T-chunks of 512: QKV-proj(c) -> RoPE(c) ->
attention(c) -> Wo-partial(c-1), with the causal mask applied as a -240
bias added into the score PSUM by the tensor engine (exp then yields ~0),
and score/PV/exp work triangularly trimmed on diagonal blocks.

All matmuls in bf16 with fp32 PSUM accumulation; softmax without max
subtraction (scores are bounded: |S/8| <= ~7), denominator fused into the
PV matmul via an appended ones-column on V.
"""

import numpy as np
import ml_dtypes

import concourse.bass as bass
import concourse.mybir as mybir
import concourse.tile as tile
from concourse import bacc
from concourse.bass_utils import run_bass_kernel_spmd
from concourse.masks import make_identity

BF16 = mybir.dt.bfloat16
F32 = mybir.dt.float32

N_CORES = 8
T = 2048
D = 2048
HD = 64          # head dim
NH = 32          # total q heads
NKV = 8          # total kv heads
NH_LOC = NH // N_CORES       # 4 q heads per core
QW = NH_LOC * HD             # 256 q dims per core
P = 128
KT = D // P                  # 16 contraction tiles
CH = 512         # T-chunk (psum free dim)
NCH = T // CH                # 4 chunks
NJ = T // P                  # 16 k-blocks
SCALE = 1.0 / 8.0            # 1/sqrt(64)
TR = T // N_CORES            # 256 output rows per core after ReduceScatter
NEG = -240.0                 # additive causal-mask bias (exp(-30) ~ 0)


def build_bass():
    nc = bacc.Bacc(None, num_devices=N_CORES)

    # I/O
    xT_d = nc.dram_tensor("xT", [D, T], BF16, kind="ExternalInput")
    w_d = nc.dram_tensor("w_all", [D, QW + 2 * HD], BF16, kind="ExternalInput")
    woT_d = nc.dram_tensor("woT", [QW, D], BF16, kind="ExternalInput")
    # extras: [R128 perm | mask bias | cos | sin] along free dim
    ex_d = nc.dram_tensor("extras", [P, 2 * P + 2 * T], BF16, kind="ExternalInput")
    y_d = nc.dram_tensor("y", [TR, D], BF16, kind="ExternalOutput")

    with tile.TileContext(nc, num_cores=N_CORES) as tc:
        _body(nc, tc, xT_d, w_d, woT_d, ex_d, y_d)
    nc.compile()
    return nc


def _body(nc, tc, xT_d, w_d, woT_d, ex_d, y_d):
    import contextlib

    ctx = contextlib.ExitStack()
    with ctx:
        consts = ctx.enter_context(tc.tile_pool(name="consts", bufs=1))
        big = ctx.enter_context(tc.tile_pool(name="big", bufs=1))
        work = ctx.enter_context(tc.tile_pool(name="work", bufs=1))
        rope_p = ctx.enter_context(tc.tile_pool(name="rope_p", bufs=2))
        yp_p = ctx.enter_context(tc.tile_pool(name="yp_p", bufs=2))
        pt_p = ctx.enter_context(tc.tile_pool(name="pt_p", bufs=3))
        rcp_p = ctx.enter_context(tc.tile_pool(name="rcp_p", bufs=2))
        psum_st = ctx.enter_context(tc.tile_pool(name="psum_st", bufs=2, space="PSUM"))
        psum_o = ctx.enter_context(tc.tile_pool(name="psum_o", bufs=2, space="PSUM"))
        dram = ctx.enter_context(tc.tile_pool(name="dram", bufs=1, space="DRAM"))

        # ---- loads: few large DMAs (per-DMA HWDGE setup ~625ns dominates
        # small transfers; bus runs at full speed on >=512B descriptors)
        w_sb = consts.tile([P, KT, QW + 2 * HD], BF16)
        w_r = w_d.rearrange("(kt p) m -> p kt m", p=P)
        nc.sync.dma_start(w_sb[:, 0:8, :], w_r[:, 0:8, :])
        xT_sb = big.tile([P, KT, T], BF16, tag="xT")
        xT_r = xT_d.rearrange("(kt p) t -> p kt t", p=P)
        nc.sync.dma_start(xT_sb[:, 0:4, 0:CH], xT_r[:, 0:4, 0:CH])
        nc.sync.dma_start(w_sb[:, 8:16, :], w_r[:, 8:16, :])
        nc.sync.dma_start(xT_sb[:, 4:16, 0:CH], xT_r[:, 4:16, 0:CH])
        for c in range(1, NCH):
            sl = slice(c * CH, (c + 1) * CH)
            nc.sync.dma_start(xT_sb[:, :, sl], xT_r[:, :, sl])
        ex_sb = consts.tile([P, 2 * P + 2 * T], BF16)
        nc.sync.dma_start(ex_sb, ex_d[:])
        R128 = ex_sb[:, 0:P]
        mbias = ex_sb[:, P : 2 * P]
        cos2 = ex_sb[:, 2 * P : 2 * P + T]
        sin2 = ex_sb[:, 2 * P + T : 2 * P + 2 * T]
        woT_sb = consts.tile([P, 2, D], BF16)
        woT_r = woT_d.rearrange("(kh p) d -> p kh d", p=P)
        nc.sync.dma_start(woT_sb, woT_r)

        ident64 = consts.tile([HD, HD], BF16)
        make_identity(nc, ident64)
        ident128 = consts.tile([P, P], BF16)
        make_identity(nc, ident128)
        ones64 = consts.tile([1, HD], BF16)
        nc.vector.memset(ones64, 1.0)

        # v with ones column appended: [tk(P), j, HD+1]
        v_aug = work.tile([P, NJ, HD + 1], BF16)
        nc.vector.memset(v_aug[:, :, HD : HD + 1], 1.0)

        projT = work.tile([P, 3, T], BF16)     # m=0: heads 0,1; m=1: heads 2,3
        qT128 = work.tile([P, 2, T], BF16)     # RoPE'd q, same packing
        # kT duplicated into both partition halves so ST lhsT base can match
        # the q operand's base for odd heads
        kT_sb = work.tile([P, T], BF16)
        OT128 = work.tile([P, 2, T], BF16)     # normalized O^T, row kh*128+p
        ypart_dram = dram.tile([T, D], BF16)
        ypart_r = ypart_dram.rearrange("(mt p) d -> p mt d", p=P)
        y_rs = dram.tile([TR, D], BF16)

        def cch(c):
            return slice(c * CH, (c + 1) * CH)

        def proj_chunk(c):
            for m in range(3):
                ps = psum_o.tile([P, CH], F32, tag="mm")
                for kt in range(KT):
                    nc.tensor.matmul(
                        ps,
                        lhsT=w_sb[:, kt, m * P : (m + 1) * P],
                        rhs=xT_sb[:, kt, cch(c)],
                        start=(kt == 0),
                        stop=(kt == KT - 1),
                    )
                nc.vector.tensor_copy(projT[:, m, cch(c)], ps)

        def rope_chunk(c):
            # q pairs: qrot = q*cos + (R q)*sin' (sign folded into sin')
            for s in range(2):
                qs = psum_o.tile([P, CH], F32, tag="mm")
                nc.tensor.matmul(
                    qs, lhsT=R128, rhs=projT[:, s, cch(c)], start=True, stop=True
                )
                t1 = rope_p.tile([P, CH], BF16, tag="t1")
                nc.vector.tensor_mul(t1, projT[:, s, cch(c)], cos2[:, cch(c)])
                t2 = rope_p.tile([P, CH], BF16, tag="t2")
                nc.vector.tensor_mul(t2, qs, sin2[:, cch(c)])
                nc.vector.tensor_add(qT128[:, s, cch(c)], t1, t2)
            # k (rows 0:64 of m=2) on the pool engine
            ks = psum_o.tile([P, CH], F32, tag="mm")
            nc.tensor.matmul(
                ks[0:HD, :],
                lhsT=R128[0:HD, 0:HD],
                rhs=projT[0:HD, 2, cch(c)],
                start=True,
                stop=True,
            )
            k1 = rope_p.tile([HD, CH], BF16, tag="k1")
            nc.gpsimd.tensor_mul(k1, projT[0:HD, 2, cch(c)], cos2[0:HD, cch(c)])
            k2 = rope_p.tile([HD, CH], BF16, tag="k2")
            nc.vector.tensor_mul(k2, ks[0:HD, :], sin2[0:HD, cch(c)])
            nc.gpsimd.tensor_add(kT_sb[0:HD, cch(c)], k1, k2)
            nc.gpsimd.tensor_copy(kT_sb[HD:P, cch(c)], kT_sb[0:HD, cch(c)])
            # v transpose for this chunk's k-blocks
            for j in range(4 * c, 4 * c + 4):
                tp = psum_o.tile([P, CH], BF16, tag="mm")
                nc.tensor.transpose(
                    tp[:, 0:HD],
                    projT[HD:P, 2, j * P : (j + 1) * P],
                    ident128[HD:P, HD:P],
                )
                nc.vector.tensor_copy(v_aug[:, j, 0:HD], tp[:, 0:HD])

        pending_norm = []

        def flush_norm(n):
            # softmax normalization: recip of denominator row, broadcast via
            # PE, multiply unnormalized O rows into OT128
            for h, c, ot in pending_norm[:n]:
                rrow = rcp_p.tile([1, CH], BF16, tag="rrow")
                with nc.allow_low_precision("softmax denom in bf16 is fine"):
                    nc.vector.reciprocal(rrow, ot[HD : HD + 1, :])
                bc = psum_o.tile([P, CH], F32, tag="mm")
                nc.tensor.matmul(
                    bc[0:HD, :], lhsT=ones64, rhs=rrow, start=True, stop=True
                )
                bcs = rcp_p.tile([HD, CH], BF16, tag="bcs")
                nc.scalar.copy(bcs, bc[0:HD, :])
                hp = (h % 2) * HD
                nc.vector.tensor_mul(
                    OT128[hp : hp + HD, h // 2, cch(c)], ot[0:HD, :], bcs
                )
            del pending_norm[:n]

        def attn_head(h, c):
            # units: [(j_or_r list, kind)] — off-diagonal pairs then the two
            # diagonal pairs; per-unit: ST (+bias on diag) -> exp -> PV.
            ot = psum_o.tile([P, CH], F32, tag="ot")
            units = []
            off = list(range(0, 4 * c))
            for g in range(0, len(off), 2):
                units.append(("off", off[g : g + 2]))
            units.append(("diag", [0, 1]))
            units.append(("diag", [2, 3]))

            hb = (h % 2) * HD   # partition base of this head's q rows

            def do_st(kind, js):
                st = psum_st.tile([P, 2, CH], F32, tag="st")
                if kind == "off":
                    for idx, j in enumerate(js):
                        nc.tensor.matmul(
                            st[:, idx, :],
                            lhsT=kT_sb[hb : hb + HD, j * P : (j + 1) * P],
                            rhs=qT128[hb : hb + HD, h // 2, cch(c)],
                            start=True,
                            stop=True,
                        )
                else:
                    for idx, r in enumerate(js):
                        j = 4 * c + r
                        q0 = P * r
                        nc.tensor.matmul(
                            st[:, idx, q0:CH],
                            lhsT=kT_sb[hb : hb + HD, j * P : (j + 1) * P],
                            rhs=qT128[
                                hb : hb + HD,
                                h // 2,
                                c * CH + q0 : (c + 1) * CH,
                            ],
                            start=True,
                            stop=False,
                            skip_group_check=True,
                        )
                        nc.tensor.matmul(
                            st[:, idx, q0 : q0 + P],
                            lhsT=ident128,
                            rhs=mbias,
                            start=False,
                            stop=True,
                            skip_group_check=True,
                        )
                return st

            def do_rest(kind, js, st):
                pt = pt_p.tile([P, 2, CH], BF16, tag="pt")
                if kind == "off":
                    nc.scalar.activation(
                        pt, st, mybir.ActivationFunctionType.Exp, scale=SCALE
                    )
                    for idx, j in enumerate(js):
                        nc.tensor.matmul(
                            ot[0 : HD + 1, :],
                            lhsT=v_aug[:, j, :],
                            rhs=pt[:, idx, :],
                            start=(j == 0),
                            stop=False,
                            skip_group_check=True,
                        )
                else:
                    for idx, r in enumerate(js):
                        j = 4 * c + r
                        q0 = P * r
                        nc.scalar.activation(
                            pt[:, idx, q0:CH],
                            st[:, idx, q0:CH],
                            mybir.ActivationFunctionType.Exp,
                            scale=SCALE,
                        )
                        nc.tensor.matmul(
                            ot[0 : HD + 1, q0:CH],
                            lhsT=v_aug[:, j, :],
                            rhs=pt[:, idx, q0:CH],
                            start=(c == 0 and r == 0),
                            stop=(r == 3),
                            skip_group_check=True,
                        )

            st_cur = do_st(*units[0])
            for u in range(len(units)):
                st_next = do_st(*units[u + 1]) if u + 1 < len(units) else None
                do_rest(units[u][0], units[u][1], st_cur)
                st_cur = st_next
            pending_norm.append((h, c, ot))

        def ypart_chunk(c):
            # y_part rows of chunk c: [512, D] = O_loc^T-slice^T @ Wo_loc^T
            ypb = yp_p.tile([P, 4, D], BF16, tag="yp")
            for i, mt in enumerate(range(4 * c, 4 * c + 4)):
                for dc in range(NCH):
                    ps = psum_o.tile([P, CH], F32, tag="mm")
                    for kh in range(2):
                        nc.tensor.matmul(
                            ps,
                            lhsT=OT128[:, kh, mt * P : (mt + 1) * P],
                            rhs=woT_sb[:, kh, cch(dc)],
                            start=(kh == 0),
                            stop=(kh == 1),
                        )
                    if dc % 2 == 0:
                        nc.vector.tensor_copy(ypb[:, i, cch(dc)], ps)
                    else:
                        nc.scalar.copy(ypb[:, i, cch(dc)], ps)
            ms = slice(4 * c, 4 * c + 4)
            nc.sync.dma_start(ypart_r[:, ms, :], ypb)
            nc.gpsimd.collective_compute(
                "ReduceScatter",
                mybir.AluOpType.add,
                replica_groups=[list(range(N_CORES))],
                ins=[ypart_dram[c * CH : (c + 1) * CH, :]],
                outs=[y_rs[c * HD : (c + 1) * HD, :]],
            )
            nc.sync.dma_start(
                y_d[c * HD : (c + 1) * HD, :], y_rs[c * HD : (c + 1) * HD, :]
            )

        for c in range(NCH):
            proj_chunk(c)
            rope_chunk(c)
            if c > 0:
                flush_norm(len(pending_norm))
                ypart_chunk(c - 1)
            for h in range(NH_LOC):
                attn_head(h, c)
                if h >= 1:
                    flush_norm(1)
        flush_norm(len(pending_norm))
        ypart_chunk(NCH - 1)


def _prep_shards(x, Wq, lora_A, lora_B, Wk, Wv, Wo):
    bf16 = ml_dtypes.bfloat16
    xT = np.ascontiguousarray(x[0].T).astype(bf16)

    theta = 1.0 / (10000.0 ** (np.arange(0, HD, 2, dtype=np.float32) / HD))
    pos = np.arange(T, dtype=np.float32)
    ang = pos[:, None] * theta[None, :]
    ang = np.concatenate([ang, ang], axis=-1)          # [T, HD]
    cosT = np.cos(ang).T                               # [HD, T]
    sinT = np.sin(ang).T
    sign = np.where(np.arange(HD) < HD // 2, -1.0, 1.0).astype(np.float32)
    sinTs = sinT * sign[:, None]
    cos2 = np.concatenate([cosT, cosT], 0)             # [128, T]
    sin2 = np.concatenate([sinTs, sinTs], 0)

    # RoPE half-rotation permutation (per 64-row head block), symmetric
    perm = np.concatenate(
        [np.arange(32, 64), np.arange(0, 32), np.arange(96, 128), np.arange(64, 96)]
    )
    R = np.eye(P, dtype=np.float32)[perm]

    # additive causal bias for the leading square of each diagonal block:
    # masked iff q_rel < k_rel i.e. f < p
    f_idx = np.arange(P)[None, :]
    p_idx = np.arange(P)[:, None]
    mb = np.where(f_idx < p_idx, NEG, 0.0).astype(np.float32)

    extras = np.ascontiguousarray(
        np.concatenate([R, mb, cos2, sin2], axis=1)
    ).astype(bf16)

    Wq_eff = Wq + lora_B.astype(np.float64) @ lora_A.astype(np.float64)
    Wq_eff = Wq_eff.astype(np.float32)

    in_maps = []
    for i in range(N_CORES):
        wq_i = Wq_eff[QW * i : QW * (i + 1), :]        # [256, D]
        wk_i = Wk[HD * i : HD * (i + 1), :]            # [64, D]
        wv_i = Wv[HD * i : HD * (i + 1), :]
        w_all = np.ascontiguousarray(
            np.concatenate([wq_i, wk_i, wv_i], 0).T
        ).astype(bf16)                                 # [D, 384]
        # Wo columns for this core's heads, transposed: [256, D]
        woT = np.ascontiguousarray(Wo[:, QW * i : QW * (i + 1)].T).astype(bf16)
        in_maps.append({
            "xT": xT,
            "w_all": w_all,
            "woT": woT,
            "extras": extras,
        })
    return in_maps


def run(inputs, trace=False, **kw):
    nc = build_bass()
    in_maps = _prep_shards(**inputs)
    res = run_bass_kernel_spmd(
        nc, in_maps, core_ids=list(range(N_CORES)), trace=trace, **kw
    )
    # core i, chunk c holds final y rows 512c + 64i .. +64
    y = np.zeros((T, D), dtype=np.float32)
    for i in range(N_CORES):
        ri = np.asarray(res.results[i]["y"]).astype(np.float32)
        for c in range(NCH):
            y[CH * c + HD * i : CH * c + HD * (i + 1)] = ri[HD * c : HD * (c + 1)]
    return y[None], res


def kernel(**inputs):
    y, _ = run(inputs)
    return y


# revision 24
# speedup vs baseline: 2.1649x; 1.0635x over previous
"""GQA attention with LoRA-Q, tensor-parallel over 8 TRN2 cores.

Sharding (per core i of 8):
  - Q heads 4i..4i+3 (256 q-dims) and KV head i (GQA: repeat_interleave maps
    q heads [4i,4i+4) exactly onto kv head i).
  - Wq (with LoRA folded: Wq_eff = Wq + lora_B @ lora_A), Wk, Wv row-sharded;
    Wo column-sharded on its input (head) dim.
  - Each core computes a full-width PARTIAL output y_part = O_loc @ Wo_loc^T
    [T, D]; per-T-chunk ReduceScatter(add) over rows gives core i rows
    512c+64i..+64 — 8x less collective traffic than AllGathering O, and the
    first three collectives overlap attention compute.

Single fused pipeline over T-chunks of 512: QKV-proj(c) -> RoPE(c) ->
attention(c) -> Wo-partial(c-1), with the causal mask applied as a -240
bias added into the score PSUM by the tensor engine (exp then yields ~0),
and score/PV/exp work triangularly trimmed on diagonal blocks.

All matmuls in bf16 with fp32 PSUM accumulation; softmax without max
subtraction (scores are bounded: |S/8| <= ~7), denominator fused into the
PV matmul via an appended ones-column on V.
"""

import numpy as np
import ml_dtypes

import concourse.bass as bass
import concourse.mybir as mybir
import concourse.tile as tile
from concourse import bacc
from concourse.bass_utils import run_bass_kernel_spmd
from concourse.masks import make_identity

BF16 = mybir.dt.bfloat16
F32 = mybir.dt.float32

N_CORES = 8
T = 2048
D = 2048
HD = 64          # head dim
NH = 32          # total q heads
NKV = 8          # total kv heads
NH_LOC = NH // N_CORES       # 4 q heads per core
QW = NH_LOC * HD             # 256 q dims per core
P = 128
KT = D // P                  # 16 contraction tiles
CH = 512         # T-chunk (psum free dim)
NCH = T // CH                # 4 chunks
NJ = T // P                  # 16 k-blocks
SCALE = 1.0 / 8.0            # 1/sqrt(64)
TR = T // N_CORES            # 256 output rows per core after ReduceScatter
NEG = -240.0                 # additive causal-mask bias (exp(-30) ~ 0)


def build_bass():
    nc = bacc.Bacc(None, num_devices=N_CORES)

    # I/O
    xT_d = nc.dram_tensor("xT", [D, T], BF16, kind="ExternalInput")
    w_d = nc.dram_tensor("w_all", [D, QW + 2 * HD], BF16, kind="ExternalInput")
    woT_d = nc.dram_tensor("woT", [QW, D], BF16, kind="ExternalInput")
    # extras: [R128 perm | mask bias | cos | sin] along free dim
    ex_d = nc.dram_tensor("extras", [P, 2 * P + 2 * T], BF16, kind="ExternalInput")
    y_d = nc.dram_tensor("y", [TR, D], BF16, kind="ExternalOutput")

    with tile.TileContext(nc, num_cores=N_CORES) as tc:
        _body(nc, tc, xT_d, w_d, woT_d, ex_d, y_d)
    nc.compile()
    return nc


def _body(nc, tc, xT_d, w_d, woT_d, ex_d, y_d):
    import contextlib

    ctx = contextlib.ExitStack()
    with ctx:
        consts = ctx.enter_context(tc.tile_pool(name="consts", bufs=1))
        big = ctx.enter_context(tc.tile_pool(name="big", bufs=1))
        work = ctx.enter_context(tc.tile_pool(name="work", bufs=1))
        rope_p = ctx.enter_context(tc.tile_pool(name="rope_p", bufs=2))
        yp_p = ctx.enter_context(tc.tile_pool(name="yp_p", bufs=2))
        pt_p = ctx.enter_context(tc.tile_pool(name="pt_p", bufs=3))
        rcp_p = ctx.enter_context(tc.tile_pool(name="rcp_p", bufs=2))
        psum_st = ctx.enter_context(tc.tile_pool(name="psum_st", bufs=2, space="PSUM"))
        psum_o = ctx.enter_context(tc.tile_pool(name="psum_o", bufs=2, space="PSUM"))
        dram = ctx.enter_context(tc.tile_pool(name="dram", bufs=1, space="DRAM"))

        # ---- loads: few large DMAs (per-DMA HWDGE setup ~625ns dominates
        # small transfers; bus runs at full speed on >=512B descriptors)
        w_sb = consts.tile([P, KT, QW + 2 * HD], BF16)
        w_r = w_d.rearrange("(kt p) m -> p kt m", p=P)
        nc.sync.dma_start(w_sb[:, 0:8, :], w_r[:, 0:8, :])
        xT_sb = big.tile([P, KT, T], BF16, tag="xT")
        xT_r = xT_d.rearrange("(kt p) t -> p kt t", p=P)
        nc.sync.dma_start(xT_sb[:, 0:4, 0:CH], xT_r[:, 0:4, 0:CH])
        nc.sync.dma_start(w_sb[:, 8:16, :], w_r[:, 8:16, :])
        nc.sync.dma_start(xT_sb[:, 4:16, 0:CH], xT_r[:, 4:16, 0:CH])
        ex_sb = consts.tile([P, 2 * P + 2 * T], BF16)
        nc.sync.dma_start(ex_sb, ex_d[:])
        for c in range(1, NCH):
            sl = slice(c * CH, (c + 1) * CH)
            nc.sync.dma_start(xT_sb[:, :, sl], xT_r[:, :, sl])
        R128 = ex_sb[:, 0:P]
        mbias = ex_sb[:, P : 2 * P]
        cos2 = ex_sb[:, 2 * P : 2 * P + T]
        sin2 = ex_sb[:, 2 * P + T : 2 * P + 2 * T]
        woT_sb = consts.tile([P, 2, D], BF16)
        woT_r = woT_d.rearrange("(kh p) d -> p kh d", p=P)
        nc.sync.dma_start(woT_sb, woT_r)

        ident64 = consts.tile([HD, HD], BF16)
        make_identity(nc, ident64)
        ident128 = consts.tile([P, P], BF16)
        make_identity(nc, ident128)
        ones64 = consts.tile([1, HD], BF16)
        nc.vector.memset(ones64, 1.0)

        # v with ones column appended: [tk(P), j, HD+1]
        v_aug = work.tile([P, NJ, HD + 1], BF16)
        nc.vector.memset(v_aug[:, :, HD : HD + 1], 1.0)

        projT = work.tile([P, 3, T], BF16)     # m=0: heads 0,1; m=1: heads 2,3
        qT128 = work.tile([P, 2, T], BF16)     # RoPE'd q, same packing
        # kT duplicated into both partition halves so ST lhsT base can match
        # the q operand's base for odd heads
        kT_sb = work.tile([P, T], BF16)
        OT128 = work.tile([P, 2, T], BF16)     # normalized O^T, row kh*128+p
        ypart_dram = dram.tile([T, D], BF16)
        ypart_r = ypart_dram.rearrange("(mt p) d -> p mt d", p=P)
        y_rs = dram.tile([TR, D], BF16)

        def cch(c):
            return slice(c * CH, (c + 1) * CH)

        def proj_chunk(c):
            for m in range(3):
                ps = psum_o.tile([P, CH], F32, tag="mm")
                for kt in range(KT):
                    nc.tensor.matmul(
                        ps,
                        lhsT=w_sb[:, kt, m * P : (m + 1) * P],
                        rhs=xT_sb[:, kt, cch(c)],
                        start=(kt == 0),
                        stop=(kt == KT - 1),
                    )
                nc.vector.tensor_copy(projT[:, m, cch(c)], ps)

        def rope_chunk(c):
            # q pairs: qrot = q*cos + (R q)*sin' (sign folded into sin')
            for s in range(2):
                qs = psum_o.tile([P, CH], F32, tag="mm")
                nc.tensor.matmul(
                    qs, lhsT=R128, rhs=projT[:, s, cch(c)], start=True, stop=True
                )
                t1 = rope_p.tile([P, CH], BF16, tag="t1")
                nc.vector.tensor_mul(t1, projT[:, s, cch(c)], cos2[:, cch(c)])
                t2 = rope_p.tile([P, CH], BF16, tag="t2")
                nc.vector.tensor_mul(t2, qs, sin2[:, cch(c)])
                nc.vector.tensor_add(qT128[:, s, cch(c)], t1, t2)
            # k (rows 0:64 of m=2) on the pool engine
            ks = psum_o.tile([P, CH], F32, tag="mm")
            nc.tensor.matmul(
                ks[0:HD, :],
                lhsT=R128[0:HD, 0:HD],
                rhs=projT[0:HD, 2, cch(c)],
                start=True,
                stop=True,
            )
            # all on DVE: the Pool queue must stay clear for collectives
            # (a waiting collective blocks every later Pool instruction)
            k1 = rope_p.tile([HD, CH], BF16, tag="k1")
            nc.vector.tensor_mul(k1, projT[0:HD, 2, cch(c)], cos2[0:HD, cch(c)])
            k2 = rope_p.tile([HD, CH], BF16, tag="k2")
            nc.vector.tensor_mul(k2, ks[0:HD, :], sin2[0:HD, cch(c)])
            nc.vector.tensor_add(kT_sb[0:HD, cch(c)], k1, k2)
            nc.vector.tensor_copy(kT_sb[HD:P, cch(c)], kT_sb[0:HD, cch(c)])
            # v transpose for this chunk's k-blocks
            for j in range(4 * c, 4 * c + 4):
                tp = psum_o.tile([P, CH], BF16, tag="mm")
                nc.tensor.transpose(
                    tp[:, 0:HD],
                    projT[HD:P, 2, j * P : (j + 1) * P],
                    ident128[HD:P, HD:P],
                )
                nc.vector.tensor_copy(v_aug[:, j, 0:HD], tp[:, 0:HD])

        pending_norm = []

        def flush_norm(n):
            # softmax normalization: recip of denominator row, broadcast via
            # PE, multiply unnormalized O rows into OT128
            for h, c, ot in pending_norm[:n]:
                rrow = rcp_p.tile([1, CH], BF16, tag="rrow")
                with nc.allow_low_precision("softmax denom in bf16 is fine"):
                    nc.vector.reciprocal(rrow, ot[HD : HD + 1, :])
                bc = psum_o.tile([P, CH], F32, tag="mm")
                nc.tensor.matmul(
                    bc[0:HD, :], lhsT=ones64, rhs=rrow, start=True, stop=True
                )
                bcs = rcp_p.tile([HD, CH], BF16, tag="bcs")
                nc.vector.tensor_copy(bcs, bc[0:HD, :])
                hp = (h % 2) * HD
                nc.vector.tensor_mul(
                    OT128[hp : hp + HD, h // 2, cch(c)], ot[0:HD, :], bcs
                )
            del pending_norm[:n]

        def attn_head(h, c):
            # units: [(j_or_r list, kind)] — off-diagonal pairs then the two
            # diagonal pairs; per-unit: ST (+bias on diag) -> exp -> PV.
            ot = psum_o.tile([P, CH], F32, tag="ot")
            units = []
            off = list(range(0, 4 * c))
            for g in range(0, len(off), 2):
                units.append(("off", off[g : g + 2]))
            units.append(("diag", [0, 1]))
            units.append(("diag", [2, 3]))

            hb = (h % 2) * HD   # partition base of this head's q rows

            def do_st(kind, js):
                st = psum_st.tile([P, 2, CH], F32, tag="st")
                if kind == "off":
                    for idx, j in enumerate(js):
                        nc.tensor.matmul(
                            st[:, idx, :],
                            lhsT=kT_sb[hb : hb + HD, j * P : (j + 1) * P],
                            rhs=qT128[hb : hb + HD, h // 2, cch(c)],
                            start=True,
                            stop=True,
                        )
                else:
                    for idx, r in enumerate(js):
                        j = 4 * c + r
                        q0 = P * r
                        nc.tensor.matmul(
                            st[:, idx, q0:CH],
                            lhsT=kT_sb[hb : hb + HD, j * P : (j + 1) * P],
                            rhs=qT128[
                                hb : hb + HD,
                                h // 2,
                                c * CH + q0 : (c + 1) * CH,
                            ],
                            start=True,
                            stop=False,
                            skip_group_check=True,
                        )
                        nc.tensor.matmul(
                            st[:, idx, q0 : q0 + P],
                            lhsT=ident128,
                            rhs=mbias,
                            start=False,
                            stop=True,
                            skip_group_check=True,
                        )
                return st

            def do_rest(kind, js, st):
                pt = pt_p.tile([P, 2, CH], BF16, tag="pt")
                if kind == "off":
                    nc.scalar.activation(
                        pt, st, mybir.ActivationFunctionType.Exp, scale=SCALE
                    )
                    for idx, j in enumerate(js):
                        nc.tensor.matmul(
                            ot[0 : HD + 1, :],
                            lhsT=v_aug[:, j, :],
                            rhs=pt[:, idx, :],
                            start=(j == 0),
                            stop=False,
                            skip_group_check=True,
                        )
                else:
                    # one exp over both blocks at the union of their valid
                    # column ranges; the stale sub-range of the second block
                    # is never read by its PV matmul
                    q0u = P * js[0]
                    nc.scalar.activation(
                        pt[:, :, q0u:CH],
                        st[:, :, q0u:CH],
                        mybir.ActivationFunctionType.Exp,
                        scale=SCALE,
                    )
                    for idx, r in enumerate(js):
                        j = 4 * c + r
                        q0 = P * r
                        nc.tensor.matmul(
                            ot[0 : HD + 1, q0:CH],
                            lhsT=v_aug[:, j, :],
                            rhs=pt[:, idx, q0:CH],
                            start=(c == 0 and r == 0),
                            stop=(r == 3),
                            skip_group_check=True,
                        )

            st_cur = do_st(*units[0])
            for u in range(len(units)):
                st_next = do_st(*units[u + 1]) if u + 1 < len(units) else None
                do_rest(units[u][0], units[u][1], st_cur)
                st_cur = st_next
            pending_norm.append((h, c, ot))

        def ypart_chunk(c):
            # y_part rows of chunk c: [512, D] = O_loc^T-slice^T @ Wo_loc^T
            ypb = yp_p.tile([P, 4, D], BF16, tag="yp")
            for i, mt in enumerate(range(4 * c, 4 * c + 4)):
                for dc in range(NCH):
                    ps = psum_o.tile([P, CH], F32, tag="mm")
                    for kh in range(2):
                        nc.tensor.matmul(
                            ps,
                            lhsT=OT128[:, kh, mt * P : (mt + 1) * P],
                            rhs=woT_sb[:, kh, cch(dc)],
                            start=(kh == 0),
                            stop=(kh == 1),
                        )
                    if dc % 2 == 0:
                        nc.vector.tensor_copy(ypb[:, i, cch(dc)], ps)
                    else:
                        nc.scalar.copy(ypb[:, i, cch(dc)], ps)
            ms = slice(4 * c, 4 * c + 4)
            nc.sync.dma_start(ypart_r[:, ms, :], ypb)
            nc.gpsimd.collective_compute(
                "ReduceScatter",
                mybir.AluOpType.add,
                replica_groups=[list(range(N_CORES))],
                ins=[ypart_dram[c * CH : (c + 1) * CH, :]],
                outs=[y_rs[c * HD : (c + 1) * HD, :]],
            )

        for c in range(NCH):
            proj_chunk(c)
            rope_chunk(c)
            if c > 0:
                flush_norm(len(pending_norm))
                ypart_chunk(c - 1)
            for h in range(NH_LOC):
                attn_head(h, c)
                if h >= 1:
                    flush_norm(1)
        flush_norm(len(pending_norm))
        ypart_chunk(NCH - 1)
        nc.sync.dma_start(y_d[:], y_rs[:])


def _prep_shards(x, Wq, lora_A, lora_B, Wk, Wv, Wo):
    bf16 = ml_dtypes.bfloat16
    xT = np.ascontiguousarray(x[0].T).astype(bf16)

    theta = 1.0 / (10000.0 ** (np.arange(0, HD, 2, dtype=np.float32) / HD))
    pos = np.arange(T, dtype=np.float32)
    ang = pos[:, None] * theta[None, :]
    ang = np.concatenate([ang, ang], axis=-1)          # [T, HD]
    cosT = np.cos(ang).T                               # [HD, T]
    sinT = np.sin(ang).T
    sign = np.where(np.arange(HD) < HD // 2, -1.0, 1.0).astype(np.float32)
    sinTs = sinT * sign[:, None]
    cos2 = np.concatenate([cosT, cosT], 0)             # [128, T]
    sin2 = np.concatenate([sinTs, sinTs], 0)

    # RoPE half-rotation permutation (per 64-row head block), symmetric
    perm = np.concatenate(
        [np.arange(32, 64), np.arange(0, 32), np.arange(96, 128), np.arange(64, 96)]
    )
    R = np.eye(P, dtype=np.float32)[perm]

    # additive causal bias for the leading square of each diagonal block:
    # masked iff q_rel < k_rel i.e. f < p
    f_idx = np.arange(P)[None, :]
    p_idx = np.arange(P)[:, None]
    mb = np.where(f_idx < p_idx, NEG, 0.0).astype(np.float32)

    extras = np.ascontiguousarray(
        np.concatenate([R, mb, cos2, sin2], axis=1)
    ).astype(bf16)

    Wq_eff = Wq + lora_B.astype(np.float64) @ lora_A.astype(np.float64)
    Wq_eff = Wq_eff.astype(np.float32)

    in_maps = []
    for i in range(N_CORES):
        wq_i = Wq_eff[QW * i : QW * (i + 1), :]        # [256, D]
        wk_i = Wk[HD * i : HD * (i + 1), :]            # [64, D]
        wv_i = Wv[HD * i : HD * (i + 1), :]
        w_all = np.ascontiguousarray(
            np.concatenate([wq_i, wk_i, wv_i], 0).T
        ).astype(bf16)                                 # [D, 384]
        # Wo columns for this core's heads, transposed: [256, D]
        woT = np.ascontiguousarray(Wo[:, QW * i : QW * (i + 1)].T).astype(bf16)
        in_maps.append({
            "xT": xT,
            "w_all": w_all,
            "woT": woT,
            "extras": extras,
        })
    return in_maps


def run(inputs, trace=False, **kw):
    nc = build_bass()
    in_maps = _prep_shards(**inputs)
    res = run_bass_kernel_spmd(
        nc, in_maps, core_ids=list(range(N_CORES)), trace=trace, **kw
    )
    # core i, chunk c holds final y rows 512c + 64i .. +64
    y = np.zeros((T, D), dtype=np.float32)
    for i in range(N_CORES):
        ri = np.asarray(res.results[i]["y"]).astype(np.float32)
        for c in range(NCH):
            y[CH * c + HD * i : CH * c + HD * (i + 1)] = ri[HD * c : HD * (c + 1)]
    return y[None], res


def kernel(**inputs):
    y, _ = run(inputs)
    return y


# revision 28
# speedup vs baseline: 2.2216x; 1.0262x over previous
"""GQA attention with LoRA-Q, tensor-parallel over 8 TRN2 cores.

Sharding (per core i of 8):
  - Q heads 4i..4i+3 (256 q-dims) and KV head i (GQA: repeat_interleave maps
    q heads [4i,4i+4) exactly onto kv head i).
  - Wq (with LoRA folded: Wq_eff = Wq + lora_B @ lora_A), Wk, Wv row-sharded;
    Wo column-sharded on its input (head) dim.
  - Each core computes a full-width PARTIAL output y_part = O_loc @ Wo_loc^T
    [T, D]; per-T-chunk ReduceScatter(add) over rows gives core i rows
    512c+64i..+64 — 8x less collective traffic than AllGathering O, and the
    first three collectives overlap attention compute.

Single fused pipeline over T-chunks of 512: QKV-proj(c) -> RoPE(c) ->
attention(c) -> Wo-partial(c-1), with the causal mask applied as a -240
bias added into the score PSUM by the tensor engine (exp then yields ~0),
and score/PV/exp work triangularly trimmed on diagonal blocks.

All matmuls in bf16 with fp32 PSUM accumulation; softmax without max
subtraction (scores are bounded: |S/8| <= ~7), denominator fused into the
PV matmul via an appended ones-column on V.
"""

import numpy as np
import ml_dtypes

import concourse.bass as bass
import concourse.mybir as mybir
import concourse.tile as tile
from concourse import bacc
from concourse.bass_utils import run_bass_kernel_spmd
from concourse.masks import make_identity

BF16 = mybir.dt.bfloat16
F32 = mybir.dt.float32

N_CORES = 8
T = 2048
D = 2048
HD = 64          # head dim
NH = 32          # total q heads
NKV = 8          # total kv heads
NH_LOC = NH // N_CORES       # 4 q heads per core
QW = NH_LOC * HD             # 256 q dims per core
P = 128
KT = D // P                  # 16 contraction tiles
CH = 512         # T-chunk (psum free dim)
NCH = T // CH                # 4 chunks
NJ = T // P                  # 16 k-blocks
SCALE = 1.0 / 8.0            # 1/sqrt(64)
TR = T // N_CORES            # 256 output rows per core after ReduceScatter
NEG = -240.0                 # additive causal-mask bias (exp(-30) ~ 0)


def build_bass():
    nc = bacc.Bacc(None, num_devices=N_CORES)

    # I/O
    xT_d = nc.dram_tensor("xT", [D, T], BF16, kind="ExternalInput")
    w_d = nc.dram_tensor("w_all", [D, QW + 2 * HD], BF16, kind="ExternalInput")
    woT_d = nc.dram_tensor("woT", [QW, D], BF16, kind="ExternalInput")
    # extras: [R128 perm | mask bias | cos | sin] along free dim
    ex_d = nc.dram_tensor("extras", [P, 2 * P + 2 * T], BF16, kind="ExternalInput")
    y_d = nc.dram_tensor("y", [TR, D], BF16, kind="ExternalOutput")

    with tile.TileContext(nc, num_cores=N_CORES) as tc:
        _body(nc, tc, xT_d, w_d, woT_d, ex_d, y_d)
    nc.compile()
    return nc


def _body(nc, tc, xT_d, w_d, woT_d, ex_d, y_d):
    import contextlib

    ctx = contextlib.ExitStack()
    with ctx:
        consts = ctx.enter_context(tc.tile_pool(name="consts", bufs=1))
        big = ctx.enter_context(tc.tile_pool(name="big", bufs=1))
        work = ctx.enter_context(tc.tile_pool(name="work", bufs=1))
        rope_p = ctx.enter_context(tc.tile_pool(name="rope_p", bufs=2))
        yp_p = ctx.enter_context(tc.tile_pool(name="yp_p", bufs=2))
        pt_p = ctx.enter_context(tc.tile_pool(name="pt_p", bufs=3))
        rcp_p = ctx.enter_context(tc.tile_pool(name="rcp_p", bufs=2))
        psum_st = ctx.enter_context(tc.tile_pool(name="psum_st", bufs=2, space="PSUM"))
        psum_o = ctx.enter_context(tc.tile_pool(name="psum_o", bufs=2, space="PSUM"))
        dram = ctx.enter_context(tc.tile_pool(name="dram", bufs=1, space="DRAM"))

        # ---- loads: few large DMAs (per-DMA HWDGE setup ~625ns dominates
        # small transfers; bus runs at full speed on >=512B descriptors)
        w_sb = consts.tile([P, KT, QW + 2 * HD], BF16)
        w_r = w_d.rearrange("(kt p) m -> p kt m", p=P)
        nc.sync.dma_start(w_sb[:, 0:8, :], w_r[:, 0:8, :])
        xT_sb = big.tile([P, KT, T], BF16, tag="xT")
        xT_r = xT_d.rearrange("(kt p) t -> p kt t", p=P)
        nc.sync.dma_start(xT_sb[:, 0:4, 0:CH], xT_r[:, 0:4, 0:CH])
        nc.sync.dma_start(w_sb[:, 8:16, :], w_r[:, 8:16, :])
        nc.sync.dma_start(xT_sb[:, 4:16, 0:CH], xT_r[:, 4:16, 0:CH])
        ex_sb = consts.tile([P, 2 * P + 2 * T], BF16)
        nc.sync.dma_start(ex_sb, ex_d[:])
        for c in range(1, NCH):
            sl = slice(c * CH, (c + 1) * CH)
            nc.sync.dma_start(xT_sb[:, :, sl], xT_r[:, :, sl])
        R128 = ex_sb[:, 0:P]
        mbias = ex_sb[:, P : 2 * P]
        cos2 = ex_sb[:, 2 * P : 2 * P + T]
        sin2 = ex_sb[:, 2 * P + T : 2 * P + 2 * T]
        woT_sb = consts.tile([P, 2, D], BF16)
        woT_r = woT_d.rearrange("(kh p) d -> p kh d", p=P)
        nc.sync.dma_start(woT_sb, woT_r)

        ident64 = consts.tile([HD, HD], BF16)
        make_identity(nc, ident64)
        ident128 = consts.tile([P, P], BF16)
        make_identity(nc, ident128)
        ones64 = consts.tile([1, HD], BF16)
        nc.vector.memset(ones64, 1.0)

        # v with ones column appended: [tk(P), j, HD+1]
        v_aug = work.tile([P, NJ, HD + 1], BF16)
        nc.vector.memset(v_aug[:, :, HD : HD + 1], 1.0)

        projT = work.tile([P, 3, T], BF16)     # m=0: heads 0,1; m=1: heads 2,3
        qT128 = work.tile([P, 2, T], BF16)     # RoPE'd q, same packing
        # kT duplicated into both partition halves so ST lhsT base can match
        # the q operand's base for odd heads
        kT_sb = work.tile([P, T], BF16)
        OT128 = work.tile([P, 2, T], BF16)     # normalized O^T, row kh*128+p
        ypart_dram = dram.tile([T, D], BF16)
        ypart_r = ypart_dram.rearrange("(mt p) d -> p mt d", p=P)
        y_rs = dram.tile([TR, D], BF16)

        def cch(c):
            return slice(c * CH, (c + 1) * CH)

        def proj_chunk(c):
            for m in range(3):
                ps = psum_o.tile([P, CH], F32, tag="mm")
                for kt in range(KT):
                    nc.tensor.matmul(
                        ps,
                        lhsT=w_sb[:, kt, m * P : (m + 1) * P],
                        rhs=xT_sb[:, kt, cch(c)],
                        start=(kt == 0),
                        stop=(kt == KT - 1),
                    )
                nc.vector.tensor_copy(projT[:, m, cch(c)], ps)

        def rope_chunk(c):
            # q pairs: qrot = q*cos + (R q)*sin' (sign folded into sin')
            for s in range(2):
                qs = psum_o.tile([P, CH], F32, tag="mm")
                nc.tensor.matmul(
                    qs, lhsT=R128, rhs=projT[:, s, cch(c)], start=True, stop=True
                )
                t1 = rope_p.tile([P, CH], BF16, tag="t1")
                nc.vector.tensor_mul(t1, projT[:, s, cch(c)], cos2[:, cch(c)])
                t2 = rope_p.tile([P, CH], BF16, tag="t2")
                nc.vector.tensor_mul(t2, qs, sin2[:, cch(c)])
                nc.vector.tensor_add(qT128[:, s, cch(c)], t1, t2)
            # k (rows 0:64 of m=2) on the pool engine
            ks = psum_o.tile([P, CH], F32, tag="mm")
            nc.tensor.matmul(
                ks[0:HD, :],
                lhsT=R128[0:HD, 0:HD],
                rhs=projT[0:HD, 2, cch(c)],
                start=True,
                stop=True,
            )
            # all on DVE: the Pool queue must stay clear for collectives
            # (a waiting collective blocks every later Pool instruction)
            k1 = rope_p.tile([HD, CH], BF16, tag="k1")
            nc.vector.tensor_mul(k1, projT[0:HD, 2, cch(c)], cos2[0:HD, cch(c)])
            k2 = rope_p.tile([HD, CH], BF16, tag="k2")
            nc.vector.tensor_mul(k2, ks[0:HD, :], sin2[0:HD, cch(c)])
            nc.vector.tensor_add(kT_sb[0:HD, cch(c)], k1, k2)
            nc.vector.tensor_copy(kT_sb[HD:P, cch(c)], kT_sb[0:HD, cch(c)])
            # v transpose for this chunk's k-blocks
            for j in range(4 * c, 4 * c + 4):
                tp = psum_o.tile([P, CH], BF16, tag="mm")
                nc.tensor.transpose(
                    tp[:, 0:HD],
                    projT[HD:P, 2, j * P : (j + 1) * P],
                    ident128[HD:P, HD:P],
                )
                nc.vector.tensor_copy(v_aug[:, j, 0:HD], tp[:, 0:HD])

        pending_norm = []

        def flush_norm(n):
            # softmax normalization: recip of denominator row, broadcast via
            # PE, multiply unnormalized O rows into OT128
            for h, c, ot in pending_norm[:n]:
                rrow = rcp_p.tile([1, CH], BF16, tag="rrow")
                with nc.allow_low_precision("softmax denom in bf16 is fine"):
                    nc.vector.reciprocal(rrow, ot[HD : HD + 1, :])
                bc = psum_o.tile([P, CH], F32, tag="mm")
                nc.tensor.matmul(
                    bc[0:HD, :], lhsT=ones64, rhs=rrow, start=True, stop=True
                )
                bcs = rcp_p.tile([HD, CH], BF16, tag="bcs")
                nc.vector.tensor_copy(bcs, bc[0:HD, :])
                hp = (h % 2) * HD
                nc.vector.tensor_mul(
                    OT128[hp : hp + HD, h // 2, cch(c)], ot[0:HD, :], bcs
                )
            del pending_norm[:n]

        def attn_head(h, c, filler):
            # units: [(j_or_r list, kind)] — off-diagonal pairs then the two
            # diagonal pairs; per-unit: ST (+bias on diag) -> exp -> PV.
            ot = psum_o.tile([P, CH], F32, tag="ot")
            units = []
            off = list(range(0, 4 * c))
            for g in range(0, len(off), 2):
                units.append(("off", off[g : g + 2]))
            units.append(("diag", [0, 1]))
            units.append(("diag", [2, 3]))

            hb = (h % 2) * HD   # partition base of this head's q rows

            def do_st(kind, js):
                st = psum_st.tile([P, 2, CH], F32, tag="st")
                if kind == "off":
                    for idx, j in enumerate(js):
                        nc.tensor.matmul(
                            st[:, idx, :],
                            lhsT=kT_sb[hb : hb + HD, j * P : (j + 1) * P],
                            rhs=qT128[hb : hb + HD, h // 2, cch(c)],
                            start=True,
                            stop=True,
                        )
                else:
                    for idx, r in enumerate(js):
                        j = 4 * c + r
                        q0 = P * r
                        nc.tensor.matmul(
                            st[:, idx, q0:CH],
                            lhsT=kT_sb[hb : hb + HD, j * P : (j + 1) * P],
                            rhs=qT128[
                                hb : hb + HD,
                                h // 2,
                                c * CH + q0 : (c + 1) * CH,
                            ],
                            start=True,
                            stop=False,
                            skip_group_check=True,
                        )
                        nc.tensor.matmul(
                            st[:, idx, q0 : q0 + P],
                            lhsT=ident128,
                            rhs=mbias,
                            start=False,
                            stop=True,
                            skip_group_check=True,
                        )
                return st

            def do_rest(kind, js, st):
                pt = pt_p.tile([P, 2, CH], BF16, tag="pt")
                if kind == "off":
                    nc.scalar.activation(
                        pt, st, mybir.ActivationFunctionType.Exp, scale=SCALE
                    )
                    for idx, j in enumerate(js):
                        nc.tensor.matmul(
                            ot[0 : HD + 1, :],
                            lhsT=v_aug[:, j, :],
                            rhs=pt[:, idx, :],
                            start=(j == 0),
                            stop=False,
                            skip_group_check=True,
                        )
                else:
                    # one exp over both blocks at the union of their valid
                    # column ranges; the stale sub-range of the second block
                    # is never read by its PV matmul
                    q0u = P * js[0]
                    nc.scalar.activation(
                        pt[:, :, q0u:CH],
                        st[:, :, q0u:CH],
                        mybir.ActivationFunctionType.Exp,
                        scale=SCALE,
                    )
                    for idx, r in enumerate(js):
                        j = 4 * c + r
                        q0 = P * r
                        nc.tensor.matmul(
                            ot[0 : HD + 1, q0:CH],
                            lhsT=v_aug[:, j, :],
                            rhs=pt[:, idx, q0:CH],
                            start=(c == 0 and r == 0),
                            stop=(r == 3),
                            skip_group_check=True,
                        )

            st_cur = do_st(*units[0])
            for u in range(len(units)):
                st_next = do_st(*units[u + 1]) if u + 1 < len(units) else None
                do_rest(units[u][0], units[u][1], st_cur)
                if filler:
                    filler.pop(0)()  # independent PE work to fill exp-wait
                st_cur = st_next
            pending_norm.append((h, c, ot))

        def ypart_fillers(c):
            # y_part rows of chunk c: [512, D] = O_loc^T-slice^T @ Wo_loc^T.
            # Returned as a list of small closures so the PE work can be
            # sprinkled between attention units (fills exp-wait bubbles).
            ypb = yp_p.tile([P, 4, D], BF16, tag="yp")

            def group(i, mt, dc):
                def emit():
                    ps = psum_o.tile([P, CH], F32, tag="mm")
                    for kh in range(2):
                        nc.tensor.matmul(
                            ps,
                            lhsT=OT128[:, kh, mt * P : (mt + 1) * P],
                            rhs=woT_sb[:, kh, cch(dc)],
                            start=(kh == 0),
                            stop=(kh == 1),
                        )
                    if dc % 2 == 0:
                        nc.vector.tensor_copy(ypb[:, i, cch(dc)], ps)
                    else:
                        nc.scalar.copy(ypb[:, i, cch(dc)], ps)

                return emit

            def dma_half(half):
                def emit():
                    ms = slice(4 * c + 2 * half, 4 * c + 2 * half + 2)
                    nc.sync.dma_start(
                        ypart_r[:, ms, :], ypb[:, 2 * half : 2 * half + 2, :]
                    )

                return emit

            def rs():
                def emit():
                    nc.gpsimd.collective_compute(
                        "ReduceScatter",
                        mybir.AluOpType.add,
                        replica_groups=[list(range(N_CORES))],
                        ins=[ypart_dram[c * CH : (c + 1) * CH, :]],
                        outs=[y_rs[c * HD : (c + 1) * HD, :]],
                    )

                return emit

            items = []
            for i, mt in enumerate(range(4 * c, 4 * c + 4)):
                for dc in range(NCH):
                    items.append(group(i, mt, dc))
                if i == 1:
                    items.append(dma_half(0))
            items.append(dma_half(1))
            items.append(rs())
            return items

        for c in range(NCH):
            proj_chunk(c)
            rope_chunk(c)
            filler = []
            if c > 0:
                flush_norm(len(pending_norm))
                filler = ypart_fillers(c - 1)
            for h in range(NH_LOC):
                attn_head(h, c, filler)
                if h >= 1:
                    flush_norm(1)
            for f in filler:
                f()
            del filler[:]
        flush_norm(len(pending_norm))
        for f in ypart_fillers(NCH - 1):
            f()
        nc.sync.dma_start(y_d[:], y_rs[:])


def _prep_shards(x, Wq, lora_A, lora_B, Wk, Wv, Wo):
    bf16 = ml_dtypes.bfloat16
    xT = np.ascontiguousarray(x[0].T).astype(bf16)

    theta = 1.0 / (10000.0 ** (np.arange(0, HD, 2, dtype=np.float32) / HD))
    pos = np.arange(T, dtype=np.float32)
    ang = pos[:, None] * theta[None, :]
    ang = np.concatenate([ang, ang], axis=-1)          # [T, HD]
    cosT = np.cos(ang).T                               # [HD, T]
    sinT = np.sin(ang).T
    sign = np.where(np.arange(HD) < HD // 2, -1.0, 1.0).astype(np.float32)
    sinTs = sinT * sign[:, None]
    cos2 = np.concatenate([cosT, cosT], 0)             # [128, T]
    sin2 = np.concatenate([sinTs, sinTs], 0)

    # RoPE half-rotation permutation (per 64-row head block), symmetric
    perm = np.concatenate(
        [np.arange(32, 64), np.arange(0, 32), np.arange(96, 128), np.arange(64, 96)]
    )
    R = np.eye(P, dtype=np.float32)[perm]

    # additive causal bias for the leading square of each diagonal block:
    # masked iff q_rel < k_rel i.e. f < p
    f_idx = np.arange(P)[None, :]
    p_idx = np.arange(P)[:, None]
    mb = np.where(f_idx < p_idx, NEG, 0.0).astype(np.float32)

    extras = np.ascontiguousarray(
        np.concatenate([R, mb, cos2, sin2], axis=1)
    ).astype(bf16)

    Wq_eff = Wq + lora_B.astype(np.float64) @ lora_A.astype(np.float64)
    Wq_eff = Wq_eff.astype(np.float32)

    in_maps = []
    for i in range(N_CORES):
        wq_i = Wq_eff[QW * i : QW * (i + 1), :]        # [256, D]
        wk_i = Wk[HD * i : HD * (i + 1), :]            # [64, D]
        wv_i = Wv[HD * i : HD * (i + 1), :]
        w_all = np.ascontiguousarray(
            np.concatenate([wq_i, wk_i, wv_i], 0).T
        ).astype(bf16)                                 # [D, 384]
        # Wo columns for this core's heads, transposed: [256, D]
        woT = np.ascontiguousarray(Wo[:, QW * i : QW * (i + 1)].T).astype(bf16)
        in_maps.append({
            "xT": xT,
            "w_all": w_all,
            "woT": woT,
            "extras": extras,
        })
    return in_maps


def run(inputs, trace=False, **kw):
    nc = build_bass()
    in_maps = _prep_shards(**inputs)
    res = run_bass_kernel_spmd(
        nc, in_maps, core_ids=list(range(N_CORES)), trace=trace, **kw
    )
    # core i, chunk c holds final y rows 512c + 64i .. +64
    y = np.zeros((T, D), dtype=np.float32)
    for i in range(N_CORES):
        ri = np.asarray(res.results[i]["y"]).astype(np.float32)
        for c in range(NCH):
            y[CH * c + HD * i : CH * c + HD * (i + 1)] = ri[HD * c : HD * (c + 1)]
    return y[None], res


def kernel(**inputs):
    y, _ = run(inputs)
    return y
